# revision 24
# baseline (speedup 1.0000x reference)
"""Trainium2 Bass kernel for nn_L1RegressionActionHead.

Data-parallel over batch: 16 batch items -> 8 cores x 2 items.
All activations are kept "dim-major" on chip: (d on partitions, tokens on
free axis), so every matmul streams with the contraction dim on partitions
and no on-chip transposes are needed.  x / output are transposed on host.

RoPE trick: q/k projection weights are column-permuted on the host so each
head's dims are de-interleaved ([0,2,..,126,1,3,..,127]).  rotate_half then
becomes a swap of the two 64-partition blocks (done with 2 SBUF->SBUF DMAs)
and the cos/sin tables become plain 2D elementwise multiplies.  The 1/sqrt(HD)
score scale is folded into the q rope tables, and sigmoid(g) is folded into
the k_task rope tables (host-computed, tables are kernel inputs).

Softmax: scores are tiny (|s| < ~3) so exp is applied without max
subtraction (mathematically identical); the denominator is computed with a
ones-vector matmul over the 66 key partitions and applied via a
partition-broadcast multiply.
"""

import math
import sys

import numpy as np

sys.path.insert(0, "/opt/trn_rl_repo")

import ml_dtypes  # noqa: E402

import concourse.bass as bass  # noqa: E402
import concourse.tile as tile  # noqa: E402
from concourse import bacc, mybir  # noqa: E402
from concourse.bass_utils import run_bass_kernel_spmd  # noqa: E402

BF16 = ml_dtypes.bfloat16
F32 = mybir.dt.float32
BF = mybir.dt.bfloat16
AF = mybir.ActivationFunctionType
OP = mybir.AluOpType

DIM = 1024
HEADS = 8
HD = 128
B = 16
T = 1024
KT = 64
KA = 2
KV = KT + KA  # 66
LN_EPS = 1e-5
NCORES = 8
BPC = B // NCORES  # 2 batch items per core
P = 128
TK = DIM // P  # 8 k/d tiles
NCH = T // 512  # 2 free-dim chunks of 512 tokens

# de-interleave permutation within each head's 128 dims
_PERM_HEAD = np.concatenate([np.arange(0, HD, 2), np.arange(1, HD, 2)])
_PERM_FULL = np.concatenate([h * HD + _PERM_HEAD for h in range(HEADS)])

# weight order inside the "wcat" input tensor
_WIDX = {"w_qa": 0, "w_qt": 1, "w_ka": 2, "w_kt": 3, "w_va": 4, "w_vt": 5,
         "w_o": 6, "w_ffn": 7}
# bias slots inside "bias_cat": per-partition [128, slot, ko]
_BIDX = {"b_qa": 0, "b_qt": 1, "b_ka": 2, "b_kt": 3, "b_o": 4, "b_ffn": 5,
         "ln_g": 6, "ln_b": 7}

_CACHED = None  # compiled Bass program, built once per process
LAST_RESULTS = None  # BassKernelResults of the most recent run


def _build_program():
    nc = bacc.Bacc("TRN2", target_bir_lowering=False, debug=False,
                   enable_asserts=False)

    xt_d = nc.dram_tensor("xt", (P, BPC, TK, T), BF, kind="ExternalInput").ap()
    hcat_d = nc.dram_tensor("hcat", (P, TK, 2 * KV), BF, kind="ExternalInput").ap()
    wcat_d = nc.dram_tensor("wcat", (8, P, TK, DIM), BF, kind="ExternalInput").ap()
    bias_d = nc.dram_tensor("bias_cat", (P, 8, TK), F32, kind="ExternalInput").ap()
    bv_d = nc.dram_tensor("bv_comb", (P, DIM), BF, kind="ExternalInput").ap()
    cosq_d = nc.dram_tensor("cosq", (P, T), BF, kind="ExternalInput").ap()
    sinq_d = nc.dram_tensor("sinq", (P, T), BF, kind="ExternalInput").ap()
    cosk_d = nc.dram_tensor("cosk", (P, 2 * KV), BF, kind="ExternalInput").ap()
    sink_d = nc.dram_tensor("sink", (P, 2 * KV), BF, kind="ExternalInput").ap()
    out_d = nc.dram_tensor("outt", (P, BPC, TK, T), F32, kind="ExternalOutput").ap()

    with tile.TileContext(nc) as tc:
        _trace(nc, tc, xt_d, hcat_d, wcat_d, bias_d, bv_d,
               cosq_d, sinq_d, cosk_d, sink_d, out_d)
    nc.compile()
    return nc


def _trace(nc, tc, xt_d, hcat_d, wcat_d, bias_d, bv_d,
           cosq_d, sinq_d, cosk_d, sink_d, out_d):
    import contextlib
    ctx = contextlib.ExitStack()
    with ctx:
        consts = ctx.enter_context(tc.tile_pool(name="consts", bufs=1))
        acts = ctx.enter_context(tc.tile_pool(name="acts", bufs=1))
        qpool = ctx.enter_context(tc.tile_pool(name="qpool", bufs=4))
        wpool = ctx.enter_context(tc.tile_pool(name="wpool", bufs=2))
        scr = ctx.enter_context(tc.tile_pool(name="scr", bufs=2))
        scr3 = ctx.enter_context(tc.tile_pool(name="scr3", bufs=3))
        stats = ctx.enter_context(tc.tile_pool(name="stats", bufs=1))
        dscr = ctx.enter_context(tc.tile_pool(name="dscr", bufs=3, space="DRAM"))
        psum = ctx.enter_context(tc.tile_pool(name="psum", bufs=8, space="PSUM"))

        def bcast(dst_sb, src_sb, width, dtype, nm):
            # broadcast (1, width) -> (128, width) via DRAM bounce (SBUF
            # sources cannot have a zero partition step, DRAM can)
            dt_ = dscr.tile([1, width], dtype, tag=nm, name=nm)
            nc.sync.dma_start(dt_[:], src_sb)
            nc.sync.dma_start(
                dst_sb, bass.AP(tensor=dt_.tensor, offset=dt_.offset,
                                ap=[[0, P]] + list(dt_.ap[1:])))

        # ---- constants -------------------------------------------------
        bias_sb = consts.tile([P, 8, TK], F32, tag="bias")
        nc.sync.dma_start(bias_sb[:], bias_d[:])
        bv_sb = consts.tile([P, DIM], BF, tag="bv")
        nc.sync.dma_start(bv_sb[:], bv_d[:])
        cosq_sb = consts.tile([P, T], BF, tag="cosq")
        nc.sync.dma_start(cosq_sb[:], cosq_d[:])
        sinq_sb = consts.tile([P, T], BF, tag="sinq")
        nc.sync.dma_start(sinq_sb[:], sinq_d[:])
        cosk_sb = consts.tile([P, 2 * KV], BF, tag="cosk")
        nc.sync.dma_start(cosk_sb[:], cosk_d[:])
        sink_sb = consts.tile([P, 2 * KV], BF, tag="sink")
        nc.sync.dma_start(sink_sb[:], sink_d[:])
        hcat_sb = consts.tile([P, TK, 2 * KV], BF, tag="hcat")
        nc.sync.dma_start(hcat_sb[:], hcat_d[:])
        ones_sb = consts.tile([P, 1], BF, tag="ones")
        nc.vector.memset(ones_sb[:], 1.0)
        ones_mat = consts.tile([P, P], BF, tag="onesm")
        nc.vector.memset(ones_mat[:], 1.0)
        eps_sb = consts.tile([P, 1], F32, tag="eps")
        nc.vector.memset(eps_sb[:], LN_EPS)

        # ---- x (dim-major, bf16) --------------------------------------
        xt_sb = acts.tile([P, BPC, TK, T], BF, tag="xt")
        for b in range(BPC):
            for ko in range(TK):
                nc.sync.dma_start(xt_sb[:, b, ko, :], xt_d[:, b, ko, :])

        def load_w(wname):
            wt = wpool.tile([P, TK, DIM], BF, tag="w")
            for ko in range(TK):
                nc.sync.dma_start(wt[:, ko, :], wcat_d[_WIDX[wname], :, ko, :])
            return wt

        def bias_ap(bname, n):
            return bias_sb[:, _BIDX[bname], n:n + 1]

        def rope(dst, tmp_tag, cos_sb, sin_sb, width):
            # dst: (128, width) bf16, pre-rope values; in-place rotate
            sw = (scr if width > 256 else scr3).tile([P, width], BF,
                                                     tag=tmp_tag, name=tmp_tag)
            nc.sync.dma_start(sw[0:64, :], dst[64:128, :])
            nc.sync.dma_start(sw[64:128, :], dst[0:64, :])
            nc.vector.tensor_mul(dst, dst, cos_sb)
            nc.vector.tensor_mul(sw[:], sw[:], sin_sb)
            nc.vector.tensor_add(dst, dst, sw[:])

        # ================= Q projections + rope ========================
        # Both q weights stay resident so the qbuf allocation order is
        # qa[0], qt[0], qa[1], qt[1] -> later y/yn allocs rotate into the
        # slots of q tiles that are already dead.
        wq = {0: load_w("w_qa"), 1: load_w("w_qt")}
        q_rot = {}  # (qi, b) -> (128, TK, T) bf16 tile
        for b in range(BPC):
            for qi, bname in ((0, "b_qa"), (1, "b_qt")):
                qt_t = qpool.tile([P, TK, T], BF, tag="qbuf",
                                  name=f"q{qi}_{b}")
                q_rot[(qi, b)] = qt_t
                for n in range(TK):
                    for c in range(NCH):
                        ps = psum.tile([P, 512], F32, tag="ps")
                        for k in range(TK):
                            nc.tensor.matmul(
                                ps[:], wq[qi][:, k, n * P:(n + 1) * P],
                                xt_sb[:, b, k, c * 512:(c + 1) * 512],
                                start=(k == 0), stop=(k == TK - 1))
                        nc.scalar.activation(
                            qt_t[:, n, c * 512:(c + 1) * 512], ps[:],
                            AF.Identity, bias=bias_ap(bname, n), scale=1.0)
                    rope(qt_t[:, n, :], "qsw", cosq_sb[:], sinq_sb[:], T)

        # ================= K projections + rope ========================
        # krot columns: [0:64]=task b0, [64:128]=task b1, [128:130]=ad b0,
        # [130:132]=ad b1
        krot = acts.tile([P, TK, 2 * KV], BF, tag="krot")
        wt = load_w("w_kt")
        for n in range(TK):
            ps = psum.tile([P, 512], F32, tag="ps")
            for k in range(TK):
                nc.tensor.matmul(ps[:, 0:128], wt[:, k, n * P:(n + 1) * P],
                                 hcat_sb[:, k, 0:128],
                                 start=(k == 0), stop=(k == TK - 1))
            nc.scalar.activation(krot[:, n, 0:128], ps[:, 0:128],
                                 AF.Identity, bias=bias_ap("b_kt", n), scale=1.0)
        wt = load_w("w_ka")
        for n in range(TK):
            ps = psum.tile([P, 512], F32, tag="ps")
            for k in range(TK):
                nc.tensor.matmul(ps[:, 0:4], wt[:, k, n * P:(n + 1) * P],
                                 hcat_sb[:, k, 128:132],
                                 start=(k == 0), stop=(k == TK - 1))
            nc.scalar.activation(krot[:, n, 128:132], ps[:, 0:4],
                                 AF.Identity, bias=bias_ap("b_ka", n), scale=1.0)
        for n in range(TK):
            rope(krot[:, n, :], "ksw", cosk_sb[:], sink_sb[:], 2 * KV)

        # ================= V projections (token-major) =================
        # vcomb rows: [0:64]=task tokens, [64:66]=adapter tokens
        vcomb = acts.tile([P, BPC, DIM], BF, tag="vcomb")
        wt = load_w("w_vt")
        for b in range(BPC):
            for c in range(NCH):
                ps = psum.tile([P, 512], F32, tag="ps")
                for k in range(TK):
                    nc.tensor.matmul(ps[0:64, :],
                                     hcat_sb[:, k, b * 64:(b + 1) * 64],
                                     wt[:, k, c * 512:(c + 1) * 512],
                                     start=(k == 0), stop=(k == TK - 1))
                nc.vector.tensor_add(vcomb[0:64, b, c * 512:(c + 1) * 512],
                                     ps[0:64, :],
                                     bv_sb[0:64, c * 512:(c + 1) * 512])
        wt = load_w("w_va")
        for b in range(BPC):
            for c in range(NCH):
                ps = psum.tile([P, 512], F32, tag="ps")
                for k in range(TK):
                    nc.tensor.matmul(ps[64:66, :],
                                     hcat_sb[:, k, 128 + 2 * b:130 + 2 * b],
                                     wt[:, k, c * 512:(c + 1) * 512],
                                     start=(k == 0), stop=(k == TK - 1))
                nc.vector.tensor_add(vcomb[64:66, b, c * 512:(c + 1) * 512],
                                     ps[64:66, :],
                                     bv_sb[64:66, c * 512:(c + 1) * 512])

        wt_o = load_w("w_o")
        wt_f = load_w("w_ffn")

        # ============ per-batch: attention -> o_proj -> LN -> FFN ======
        for b in range(BPC):
            # ---- attention ----
            attn = acts.tile([P, TK, T], BF, tag="attn")
            for h in range(HEADS):
                for c in range(NCH):
                    cs = slice(c * 512, (c + 1) * 512)
                    scps = psum.tile([P, 512], F32, tag="ps")
                    nc.tensor.matmul(scps[0:64, :], krot[:, h, b * 64:(b + 1) * 64],
                                     q_rot[(1, b)][:, h, cs], start=True, stop=True)
                    nc.tensor.matmul(scps[64:66, :],
                                     krot[:, h, 128 + 2 * b:130 + 2 * b],
                                     q_rot[(0, b)][:, h, cs], start=True, stop=True)
                    ex = scr3.tile([P, 512], BF, tag="ex")
                    nc.scalar.activation(ex[0:KV, :], scps[0:KV, :], AF.Exp)
                    # denominator pre-broadcast across 128 partitions via a
                    # ones-matrix matmul (same PE cost); 1/d = exp(-ln d) on
                    # ScalarE — DVE reciprocal is 8 cyc/column and was the
                    # kernel bottleneck (3.5us per 512-col vector)
                    dnps = psum.tile([P, 512], F32, tag="ps")
                    nc.tensor.matmul(dnps[:], ones_mat[0:KV, :],
                                     ex[0:KV, :], start=True, stop=True)
                    outps = psum.tile([P, 512], F32, tag="ps")
                    nc.tensor.matmul(outps[:], vcomb[0:KV, b, h * P:(h + 1) * P],
                                     ex[0:KV, :], start=True, stop=True)
                    lnd = scr3.tile([P, 512], F32, tag="lnd")
                    nc.scalar.activation(lnd[:], dnps[:], AF.Ln)
                    rcb = scr.tile([P, 512], BF, tag="rcb")
                    nc.scalar.activation(rcb[:], lnd[:], AF.Exp, scale=-1.0)
                    nc.vector.tensor_mul(attn[:, h, cs], outps[:], rcb[:])

            # ---- o_proj + residual ----
            y = qpool.tile([P, TK, T], BF, tag="qbuf", name=f"y{b}")
            for n in range(TK):
                for c in range(NCH):
                    cs = slice(c * 512, (c + 1) * 512)
                    ps = psum.tile([P, 512], F32, tag="ps")
                    for k in range(TK):
                        nc.tensor.matmul(ps[:], wt_o[:, k, n * P:(n + 1) * P],
                                         attn[:, k, cs],
                                         start=(k == 0), stop=(k == TK - 1))
                    osb = scr3.tile([P, 512], BF, tag="osb")
                    nc.scalar.activation(osb[:], ps[:], AF.Identity,
                                         bias=bias_ap("b_o", n), scale=1.0)
                    nc.vector.tensor_add(y[:, n, cs], osb[:], xt_sb[:, b, n, cs])

            # ---- layernorm ----
            sps = [psum.tile([P, 512], F32, tag="ps", name=f"sps{b}_{i}")
                   for i in range(NCH)]
            qps = [psum.tile([P, 512], F32, tag="ps", name=f"qps{b}_{i}")
                   for i in range(NCH)]
            for n in range(TK):
                ysq = scr.tile([P, T], BF, tag="qsw", name="ysq")
                nc.vector.tensor_mul(ysq[:], y[:, n, :], y[:, n, :])
                for c in range(NCH):
                    cs = slice(c * 512, (c + 1) * 512)
                    nc.tensor.matmul(sps[c][:], ones_mat[:],
                                     y[:, n, cs],
                                     start=(n == 0), stop=(n == TK - 1),
                                     skip_group_check=True)
                    nc.tensor.matmul(qps[c][:], ones_mat[:],
                                     ysq[:, cs],
                                     start=(n == 0), stop=(n == TK - 1),
                                     skip_group_check=True)
            # sums land pre-broadcast on all 128 partitions (ones-matrix
            # lhsT); rstd = exp(-0.5*ln(var+eps)) on ScalarE
            rstd_bc = scr.tile([P, T], BF, tag="rstdbc", bufs=1)
            c_bc = scr.tile([P, T], BF, tag="cbc", bufs=1)
            for c in range(NCH):
                cs = slice(c * 512, (c + 1) * 512)
                mu_c = scr3.tile([P, 512], BF, tag="mu_c")
                nc.vector.tensor_scalar_mul(mu_c[:], sps[c][:], 1.0 / DIM)
                vq_c = scr3.tile([P, 512], F32, tag="vq_c")
                nc.vector.tensor_scalar_mul(vq_c[:], qps[c][:], 1.0 / DIM)
                t2 = scr3.tile([P, 512], BF, tag="t2_c")
                nc.vector.tensor_mul(t2[:], mu_c[:], mu_c[:])
                nc.vector.tensor_tensor(vq_c[:], vq_c[:], t2[:], OP.subtract)
                lnv = scr3.tile([P, 512], F32, tag="lnd", name="lnv")
                nc.scalar.activation(lnv[:], vq_c[:], AF.Ln, bias=eps_sb[:],
                                     scale=1.0)
                nc.scalar.activation(rstd_bc[:, cs], lnv[:], AF.Exp,
                                     scale=-0.5)
                nc.vector.tensor_mul(c_bc[:, cs], mu_c[:], rstd_bc[:, cs])
            yn = qpool.tile([P, TK, T], BF, tag="qbuf", name=f"yn{b}")
            for n in range(TK):
                nc.vector.tensor_mul(yn[:, n, :], y[:, n, :], rstd_bc[:])
                nc.vector.tensor_tensor(yn[:, n, :], yn[:, n, :], c_bc[:],
                                        OP.subtract)
                nc.vector.tensor_scalar(yn[:, n, :], yn[:, n, :],
                                        bias_ap("ln_g", n), bias_ap("ln_b", n),
                                        OP.mult, OP.add)

            # ---- FFN + relu + store ----
            for n in range(TK):
                for c in range(NCH):
                    cs = slice(c * 512, (c + 1) * 512)
                    ps = psum.tile([P, 512], F32, tag="ps")
                    for k in range(TK):
                        nc.tensor.matmul(ps[:], wt_f[:, k, n * P:(n + 1) * P],
                                         yn[:, k, cs],
                                         start=(k == 0), stop=(k == TK - 1))
                    ob = scr3.tile([P, 512], F32, tag="ob")
                    nc.scalar.activation(ob[:], ps[:], AF.Relu,
                                         bias=bias_ap("b_ffn", n), scale=1.0)
                    nc.sync.dma_start(out_d[:, b, n, cs], ob[:])


# =====================  host-side preparation  =========================

def _rope_tables(L):
    inv = 1.0 / (10000.0 ** (np.arange(0, HD, 2, dtype=np.float32) / HD))
    freqs = np.arange(L, dtype=np.float32)[:, None] * inv[None, :]
    emb = np.concatenate([freqs, freqs], axis=-1)  # (L, 128)
    return np.cos(emb), np.sin(emb)


def _perm_tables(L, scale):
    cos, sin = _rope_tables(L)  # (L, 128)
    sign = np.concatenate([-np.ones(64, np.float32), np.ones(64, np.float32)])
    cosP = (cos[:, _PERM_HEAD].T * scale).astype(np.float32)      # (128, L)
    sinN = (sin[:, _PERM_HEAD].T * sign[:, None] * scale).astype(np.float32)
    return cosP, sinN


def _w_sb(w, permute):
    # (1024 k, 1024 n) -> (128 p, 8 ko, 1024 n) bf16, optional column perm
    if permute:
        w = w[:, _PERM_FULL]
    return np.ascontiguousarray(
        w.reshape(TK, P, DIM).transpose(1, 0, 2)).astype(BF16)


def _b_slot(bvec, permute):
    if permute:
        bvec = bvec[_PERM_FULL]
    return bvec.reshape(TK, P).T  # (128, 8)


def kernel(**inputs):
    global _CACHED
    if _CACHED is None:
        _CACHED = _build_program()
    nc = _CACHED

    inp = {k: np.asarray(v) for k, v in inputs.items()}
    x = inp["x"].astype(np.float32)
    h_a = inp["h_a"].astype(np.float32)
    h_t = inp["h_t"].astype(np.float32)
    p_in = inp["p"].astype(np.float32)
    ratio = 1.0 / (1.0 + np.exp(-np.float32(inp["g"][0])))  # sigmoid

    # weights (shared across cores)
    wcat = np.stack([
        _w_sb(inp["w_qa"], True), _w_sb(inp["w_qt"], True),
        _w_sb(inp["w_ka"], True), _w_sb(inp["w_kt"], True),
        _w_sb(inp["w_va"], False), _w_sb(inp["w_vt"], False),
        _w_sb(inp["w_o"], False), _w_sb(inp["w_ffn"], False)])
    bias_cat = np.stack([
        _b_slot(inp["b_qa"], True), _b_slot(inp["b_qt"], True),
        _b_slot(inp["b_ka"], True), _b_slot(inp["b_kt"], True),
        _b_slot(inp["b_o"], False), _b_slot(inp["b_ffn"], False),
        _b_slot(inp["ln_g"], False), _b_slot(inp["ln_b"], False)],
        axis=1).astype(np.float32)  # (128, 8slots, 8ko)
    bv_comb = np.zeros((P, DIM), np.float32)
    bv_comb[0:KT, :] = inp["b_vt"][None, :]
    bv_comb[KT:KV, :] = inp["b_va"][None, :]
    bv_comb = bv_comb.astype(BF16)

    cosq, sinq = _perm_tables(T, np.float32(1.0 / math.sqrt(HD)))
    coskt, sinkt = _perm_tables(KT, ratio)
    coska, sinka = _perm_tables(KA, np.float32(1.0))
    cosk = np.concatenate([coskt, coskt, coska, coska], axis=1)  # (128, 132)
    sink = np.concatenate([sinkt, sinkt, sinka, sinka], axis=1)

    shared = {
        "wcat": wcat, "bias_cat": bias_cat, "bv_comb": bv_comb,
        "cosq": cosq.astype(BF16), "sinq": sinq.astype(BF16),
        "cosk": cosk.astype(BF16), "sink": sink.astype(BF16),
    }

    in_maps = []
    for core in range(NCORES):
        b0 = core * BPC
        xc = x[b0:b0 + BPC]  # (2, 1024, 1024)
        xt = np.ascontiguousarray(
            xc.reshape(BPC, T, TK, P).transpose(3, 0, 2, 1)).astype(BF16)
        hcat = np.zeros((P, TK, 2 * KV), np.float32)
        for b in range(BPC):
            htT = h_t[b0 + b].T.reshape(TK, P, KT).transpose(1, 0, 2)
            hcat[:, :, b * KT:(b + 1) * KT] = htT
            had = np.stack([h_a[b0 + b, 0], p_in[b0 + b, 0]], axis=1)  # (1024,2)
            hcat[:, :, 2 * KT + b * KA:2 * KT + (b + 1) * KA] = (
                had.reshape(TK, P, KA).transpose(1, 0, 2))
        in_maps.append({"xt": xt, "hcat": hcat.astype(BF16), **shared})

    res = run_bass_kernel_spmd(nc, in_maps, core_ids=list(range(NCORES)))
    global LAST_RESULTS
    LAST_RESULTS = res

    out = np.empty((B, T, DIM), np.float32)
    for core in range(NCORES):
        ot = res.results[core]["outt"]  # (128, 2, 8, 1024)
        out[core * BPC:(core + 1) * BPC] = (
            ot.transpose(1, 3, 2, 0).reshape(BPC, T, DIM))
    return out
